# revision 17
# baseline (speedup 1.0000x reference)
"""Trainium2 Bass kernel for the MFPA attention module — v3.

Reference computation (per batch b, with N = H*W = 4096 spatial sites):
    q = Wq @ x_RGB + bq            (CQK=16 channels)
    k = Wk @ x    + bk             (bk drops out of softmax)
    v = Wv @ x    + bv             (C=64 channels)
    energy[i,j] = q_i . k_j
    att = softmax(energy, axis=j)
    out[c,i] = sum_j v[c,j] att[i,j]
    y = lam * out + x

Device strategy (8 NeuronCores): data-parallel over batch (4) x query-row
halves (2).  Per core: 2048 queries x 4096 keys.

Measured TRN2 facts this version is built on:
  - A 512-col matmul issues at ~215 ns (1 col/cycle @2.4 GHz) regardless of
    M or accumulation, PROVIDED the moving data is <=128 bytes per column.
    bf16 K=64 and fp8 K=128 both qualify; bf16 K=128 runs at half rate.
  - LDWEIGHTS is fully hidden behind the previous matmul.
  - exp on the Activation engine costs ~0.83ns/col + ~290ns/instruction, so
    it is split with the Vector engine (one-pass Schraudolph: uint8(a*e+b)
    bytes are exactly fp8e4m3 exp(e - E0); f32->uint8 saturation turns
    tiny weights into +0.0, and hardware rounds to nearest).

Pipeline: energy = qk^T xf in bf16 (K=64, M-folded weights); p = exp in
fp8e4m3; PV in fp8 (K=128 j-block, M=65 with the row-sum ones column).
Loop is j-block-outer over chunk PAIRS so each xf/v stationary serves 2
moving passes and the first pair's epilogue overlaps the second pair.
"""

import ml_dtypes
import numpy as np

import concourse.bass as bass
import concourse.mybir as mybir
import concourse.tile as tile_mod
from concourse.vector_clock import ScopedClock

B, C, HH, WW = 4, 64, 64, 64
N = HH * WW          # 4096 spatial sites
NI = N // 2          # query rows per core
CHUNK = 512          # query columns per matmul pass
NCH = NI // CHUNK    # 4 chunks (2 half-passes of 2)
JB = 128             # j-block (keys per PV matmul)
NJB = N // JB        # 32 j-blocks

V_SCALE = 16.0       # v' = 16*lam*v  (removed via 1/16 in ones vector)
E0 = 2.5             # softmax shift: p = exp(e - E0) keeps p < fp8 max
A_SCH = 8.0 * np.log2(np.e)          # 11.5416 fp8e4m3 bytes per nat
C_SCH = -0.45                        # Schraudolph rounding correction

F32 = mybir.dt.float32
F32R = mybir.dt.float32r
BF16 = mybir.dt.bfloat16
F8 = mybir.dt.float8e4
U8 = mybir.dt.uint8
EXP = mybir.ActivationFunctionType.Exp
COPY = mybir.ActivationFunctionType.Copy
MULT = mybir.AluOpType.mult
ADD = mybir.AluOpType.add

NCORES = 8


def _patched_drain_and_barrier(self, tick_clock, wait_clock):
    # The walrus build in this container rejects instructions with more than
    # one sync-wait command ("Too many sync wait commands" on the Tile tail
    # drain).  Split the aggregated drain into one drain per semaphore wait.
    nc = self.nc
    drain_inst = nc.sync.drain()
    wait_clock.add_sem_waits(
        drain_inst.ins, ScopedClock({None: tick_clock.global_clock})
    )
    inst = drain_inst.ins
    si = inst.sync_info
    waits = list(si.on_wait or []) if si else []
    if len(waits) > 1:
        si.on_wait = waits[:1]
        for w in waits[1:]:
            extra = nc.sync.drain()
            extra.ins.sync_info = mybir.SyncInfo(on_wait=[w], on_update=[])
    nc.all_engine_barrier()
    popped = nc._tile_sem_poison_stack.pop()
    assert popped is self._sem_poison
    nc.clear_and_free_semaphores(list(self.sems.allocated().values()))
    nc.all_engine_barrier()


tile_mod.TileContext._drain_and_barrier = _patched_drain_and_barrier


def _split_multi_waits(nc):
    # This walrus build accepts at most one sync-wait command per TPB
    # instruction.  Hoist extra waits onto engine NoOps placed just before
    # the instruction (engine executes in order, so semantics are kept).
    for blk in nc.m.functions[0].blocks:
        insts = list(blk.instructions)
        out = []
        changed = False
        for inst in insts:
            si = inst.sync_info
            if si is not None and si.on_wait and len(si.on_wait) > 1:
                waits = list(si.on_wait)
                si.on_wait = waits[-1:]
                for w in waits[:-1]:
                    nop = mybir.InstNoOp(name=nc.get_next_instruction_name())
                    nop.engine = inst.engine
                    nop.sync_info = mybir.SyncInfo(on_wait=[w], on_update=[])
                    out.append(nop)
                changed = True
            out.append(inst)
        if changed:
            blk.instructions = out


def build_bass(split_waits=True):
    nc = bass.Bass()
    xf = nc.declare_dram_parameter("xf", [65, N], BF16, isOutput=False)
    xq = nc.declare_dram_parameter("xq", [65, NI], BF16, isOutput=False)
    m = nc.declare_dram_parameter("m", [65, C], BF16, isOutput=False)
    wv = nc.declare_dram_parameter("wv", [65, 65], BF16, isOutput=False)
    xres = nc.declare_dram_parameter("xres", [C, NI], F32, isOutput=False)
    onesv = nc.declare_dram_parameter("onesv", [1, C], F32R, isOutput=False)
    y = nc.declare_dram_parameter("y", [C, NI], F32, isOutput=True)

    with tile_mod.TileContext(nc) as tc:
        with (
            tc.tile_pool(name="singles", bufs=1) as singles,
            tc.tile_pool(name="ppool", bufs=3) as ppool,
            tc.tile_pool(name="ypool", bufs=2) as ypool,
            tc.tile_pool(name="small", bufs=4) as small,
            tc.tile_pool(name="ps_et", bufs=2, space="PSUM") as ps_et,
            tc.tile_pool(name="ps_pv", bufs=2, space="PSUM") as ps_pv,
            tc.tile_pool(name="ps_sm", bufs=2, space="PSUM") as ps_sm,
        ):
            # ---- load constants and inputs -------------------------------
            m_sb = singles.tile([65, C], BF16)
            nc.gpsimd.dma_start(out=m_sb, in_=m[:, :])
            xq_sb = singles.tile([65, NI], BF16)
            for k in range(NCH):
                ks = slice(k * CHUNK, (k + 1) * CHUNK)
                nc.gpsimd.dma_start(out=xq_sb[:, ks], in_=xq[:, ks])
            wv_sb = singles.tile([65, 65], BF16)
            nc.gpsimd.dma_start(out=wv_sb, in_=wv[:, :])
            xf_sb = singles.tile([65, N], BF16)
            for k in range(8):
                ks = slice(k * (N // 8), (k + 1) * (N // 8))
                nc.sync.dma_start(out=xf_sb[:, ks], in_=xf[:, ks])
            xres_sb = singles.tile([C, NI], F32)
            for k in range(NCH):
                ks = slice(k * CHUNK, (k + 1) * CHUNK)
                nc.gpsimd.dma_start(out=xres_sb[:, ks], in_=xres[:, ks])
            ones_sb = singles.tile([1, C], F32R)
            nc.gpsimd.dma_start(out=ones_sb, in_=onesv[:, :])
            ebias = singles.tile([128, 1], F32)
            nc.gpsimd.memset(ebias, -E0)

            # ---- qk prep: qk = M^T xq + bqk (bf16) -----------------------
            qk_bf = singles.tile([C, NCH, CHUNK], BF16)
            for ch in range(NCH):
                isl = slice(ch * CHUNK, (ch + 1) * CHUNK)
                qs = ps_sm.tile([C, CHUNK], F32, tag="sm")
                nc.tensor.matmul(
                    out=qs, lhsT=m_sb, rhs=xq_sb[:, isl], start=True, stop=True
                )
                nc.scalar.activation(out=qk_bf[:, ch, :], in_=qs, func=COPY)

            # ---- V prep: v'[j, c] fp8, col 64 = row-sum ones -------------
            v_f8 = singles.tile([JB, NJB, 65], F8)
            for jp in range(NJB // 2):
                vp = ps_et.tile([JB, 2, CHUNK], F32, tag="e")
                for g in range(2):
                    jb = 2 * jp + g
                    nc.tensor.matmul(
                        out=vp[:, g, 0:65],
                        lhsT=xf_sb[:, jb * JB : (jb + 1) * JB],
                        rhs=wv_sb,
                        start=True, stop=True,
                    )
                if jp % 2 == 0:
                    nc.scalar.activation(
                        out=v_f8[:, 2 * jp : 2 * jp + 2, :], in_=vp[:, :, 0:65],
                        func=COPY,
                    )
                else:
                    nc.vector.tensor_copy(
                        v_f8[:, 2 * jp : 2 * jp + 2, :], vp[:, :, 0:65]
                    )

            # ---- main: half-passes over chunk pairs, j-block outer -------
            sch_a = float(A_SCH)
            sch_b = float(56.0 - A_SCH * E0 + C_SCH)
            for half in range(2):
                pvs = []
                for cc in range(2):
                    pv = ps_pv.tile([65, CHUNK], F32, tag="pv")
                    pvs.append(pv)
                for jb in range(NJB):
                    et = ps_et.tile([JB, 2, CHUNK], F32, tag="e")
                    for cc in range(2):
                        ch = 2 * half + cc
                        nc.tensor.matmul(
                            out=et[:, cc, :],
                            lhsT=xf_sb[0:C, jb * JB : (jb + 1) * JB],
                            rhs=qk_bf[:, ch, :],
                            start=True, stop=True,
                        )
                    p_t = ppool.tile([JB, 2, CHUNK], F8)
                    if jb % 2 == 0:
                        nc.scalar.activation(
                            out=p_t, in_=et, func=EXP, bias=ebias,
                        )
                    else:
                        nc.vector.tensor_scalar(
                            out=p_t.bitcast(U8), in0=et,
                            scalar1=sch_a, scalar2=sch_b,
                            op0=MULT, op1=ADD,
                        )
                    for cc in range(2):
                        nc.tensor.matmul(
                            out=pvs[cc], lhsT=v_f8[:, jb, :], rhs=p_t[:, cc, :],
                            start=(jb == 0), stop=(jb == NJB - 1),
                        )

                # ---- epilogue: y = pv[0:64]*(1/16 (x) 1/s) + xres --------
                for cc in range(2):
                    ch = 2 * half + cc
                    isl = slice(ch * CHUNK, (ch + 1) * CHUNK)
                    pv = pvs[cc]
                    r_t = small.tile([1, CHUNK], F32R)
                    with nc.allow_low_precision(reason="softmax recip"):
                        nc.vector.reciprocal(out=r_t, in_=pv[64:65, :])
                    lrb = ps_sm.tile([C, CHUNK], F32, tag="sm")
                    nc.tensor.matmul(
                        out=lrb, lhsT=ones_sb, rhs=r_t, start=True, stop=True
                    )
                    lrb_sb = small.tile([C, CHUNK], F32)
                    nc.scalar.activation(out=lrb_sb, in_=lrb, func=COPY)
                    y_t = ypool.tile([C, CHUNK], F32)
                    nc.vector.tensor_tensor(
                        out=y_t, in0=pv[0:C, :], in1=lrb_sb, op=MULT
                    )
                    nc.gpsimd.tensor_tensor(
                        out=y_t, in0=y_t, in1=xres_sb[:, isl], op=ADD
                    )
                    nc.sync.dma_start(out=y[:, isl], in_=y_t)

    if split_waits:
        _split_multi_waits(nc)
    return nc


_CACHE = {}


def kernel(**inputs):
    x = np.ascontiguousarray(np.asarray(inputs["x"], dtype=np.float32))
    x_RGB = np.ascontiguousarray(np.asarray(inputs["x_RGB"], dtype=np.float32))
    Wq = np.asarray(inputs["Wq"], dtype=np.float32)
    bq = np.asarray(inputs["bq"], dtype=np.float32)
    Wk = np.asarray(inputs["Wk"], dtype=np.float32)
    Wv = np.asarray(inputs["Wv"], dtype=np.float32)
    bv = np.asarray(inputs["bv"], dtype=np.float32)
    lam = float(np.asarray(inputs["lam"], dtype=np.float32).reshape(-1)[0])

    M = (Wq.T.astype(np.float64) @ Wk.astype(np.float64)).astype(np.float32)
    bqk = (Wk.T.astype(np.float64) @ bq.astype(np.float64)).astype(np.float32)

    m_aug = np.zeros((65, C), np.float32)
    m_aug[:C] = M
    m_aug[C] = bqk

    # wv: [ch_in(+ones), c_out(+s-col)]: v' = 16*lam*(Wv xf + bv); col 64 = 1
    wv_aug = np.zeros((65, 65), np.float32)
    wv_aug[:C, :C] = (V_SCALE * lam) * Wv.T
    wv_aug[C, :C] = (V_SCALE * lam) * bv
    wv_aug[C, C] = 1.0

    xf3 = x.reshape(B, C, N)
    xr3 = x_RGB.reshape(B, C, N)

    if "nc" not in _CACHE:
        _CACHE["nc"] = build_bass()
    nc = _CACHE["nc"]

    in_maps = []
    for core in range(NCORES):
        b, ih = core >> 1, core & 1
        # columns permuted: own query half first (static residual slice)
        xf_aug = np.empty((65, N), np.float32)
        xf_aug[:C, :NI] = xf3[b][:, ih * NI : (ih + 1) * NI]
        xf_aug[:C, NI:] = xf3[b][:, (1 - ih) * NI : (2 - ih) * NI]
        xf_aug[C] = 1.0
        xq_aug = np.empty((65, NI), np.float32)
        xq_aug[:C] = xr3[b][:, ih * NI : (ih + 1) * NI]
        xq_aug[C] = 1.0
        in_maps.append(
            {
                "xf": xf_aug.astype(ml_dtypes.bfloat16),
                "xq": xq_aug.astype(ml_dtypes.bfloat16),
                "m": m_aug.astype(ml_dtypes.bfloat16),
                "wv": wv_aug.astype(ml_dtypes.bfloat16),
                "xres": np.ascontiguousarray(xf_aug[:C, 0:NI].astype(np.float32)),
                "onesv": np.full((1, C), 1.0 / V_SCALE, np.float32),
            }
        )

    from concourse.bass_utils import run_bass_kernel_spmd

    res = run_bass_kernel_spmd(nc, in_maps, list(range(NCORES)))

    out = np.empty((B, C, N), np.float32)
    for core in range(NCORES):
        b, ih = core >> 1, core & 1
        out[b][:, ih * NI : (ih + 1) * NI] = res.results[core]["y"]
    return out.reshape(B, C, HH, WW)


# revision 19
# speedup vs baseline: 1.0069x; 1.0069x over previous
"""Trainium2 Bass kernel for the MFPA attention module — v3.

Reference computation (per batch b, with N = H*W = 4096 spatial sites):
    q = Wq @ x_RGB + bq            (CQK=16 channels)
    k = Wk @ x    + bk             (bk drops out of softmax)
    v = Wv @ x    + bv             (C=64 channels)
    energy[i,j] = q_i . k_j
    att = softmax(energy, axis=j)
    out[c,i] = sum_j v[c,j] att[i,j]
    y = lam * out + x

Device strategy (8 NeuronCores): data-parallel over batch (4) x query-row
halves (2).  Per core: 2048 queries x 4096 keys.

Measured TRN2 facts this version is built on:
  - A 512-col matmul issues at ~215 ns (1 col/cycle @2.4 GHz) regardless of
    M or accumulation, PROVIDED the moving data is <=128 bytes per column.
    bf16 K=64 and fp8 K=128 both qualify; bf16 K=128 runs at half rate.
  - LDWEIGHTS is fully hidden behind the previous matmul.
  - exp on the Activation engine costs ~0.83ns/col + ~290ns/instruction, so
    it is split with the Vector engine (one-pass Schraudolph: uint8(a*e+b)
    bytes are exactly fp8e4m3 exp(e - E0); f32->uint8 saturation turns
    tiny weights into +0.0, and hardware rounds to nearest).

Pipeline: energy = qk^T xf in bf16 (K=64, M-folded weights); p = exp in
fp8e4m3; PV in fp8 (K=128 j-block, M=65 with the row-sum ones column).
Loop is j-block-outer over chunk PAIRS so each xf/v stationary serves 2
moving passes and the first pair's epilogue overlaps the second pair.
"""

import ml_dtypes
import numpy as np

import concourse.bass as bass
import concourse.mybir as mybir
import concourse.tile as tile_mod
from concourse.vector_clock import ScopedClock

B, C, HH, WW = 4, 64, 64, 64
N = HH * WW          # 4096 spatial sites
NI = N // 2          # query rows per core
CHUNK = 512          # query columns per matmul pass
NCH = NI // CHUNK    # 4 chunks (2 half-passes of 2)
JB = 128             # j-block (keys per PV matmul)
NJB = N // JB        # 32 j-blocks

V_SCALE = 16.0       # v' = 16*lam*v  (removed via 1/16 in ones vector)
E0 = 2.5             # softmax shift: p = exp(e - E0) keeps p < fp8 max
A_SCH = 8.0 * np.log2(np.e)          # 11.5416 fp8e4m3 bytes per nat
C_SCH = -0.45                        # Schraudolph rounding correction

F32 = mybir.dt.float32
F32R = mybir.dt.float32r
BF16 = mybir.dt.bfloat16
F8 = mybir.dt.float8e4
U8 = mybir.dt.uint8
EXP = mybir.ActivationFunctionType.Exp
COPY = mybir.ActivationFunctionType.Copy
MULT = mybir.AluOpType.mult
ADD = mybir.AluOpType.add

NCORES = 8


def _patched_drain_and_barrier(self, tick_clock, wait_clock):
    # The walrus build in this container rejects instructions with more than
    # one sync-wait command ("Too many sync wait commands" on the Tile tail
    # drain).  Split the aggregated drain into one drain per semaphore wait.
    nc = self.nc
    drain_inst = nc.sync.drain()
    wait_clock.add_sem_waits(
        drain_inst.ins, ScopedClock({None: tick_clock.global_clock})
    )
    inst = drain_inst.ins
    si = inst.sync_info
    waits = list(si.on_wait or []) if si else []
    if len(waits) > 1:
        si.on_wait = waits[:1]
        for w in waits[1:]:
            extra = nc.sync.drain()
            extra.ins.sync_info = mybir.SyncInfo(on_wait=[w], on_update=[])
    nc.all_engine_barrier()
    popped = nc._tile_sem_poison_stack.pop()
    assert popped is self._sem_poison
    nc.clear_and_free_semaphores(list(self.sems.allocated().values()))
    nc.all_engine_barrier()


tile_mod.TileContext._drain_and_barrier = _patched_drain_and_barrier


def _split_multi_waits(nc):
    # This walrus build accepts at most one sync-wait command per TPB
    # instruction.  Hoist extra waits onto engine NoOps placed just before
    # the instruction (engine executes in order, so semantics are kept).
    for blk in nc.m.functions[0].blocks:
        insts = list(blk.instructions)
        out = []
        changed = False
        for inst in insts:
            si = inst.sync_info
            if si is not None and si.on_wait and len(si.on_wait) > 1:
                waits = list(si.on_wait)
                si.on_wait = waits[-1:]
                for w in waits[:-1]:
                    nop = mybir.InstNoOp(name=nc.get_next_instruction_name())
                    nop.engine = inst.engine
                    nop.sync_info = mybir.SyncInfo(on_wait=[w], on_update=[])
                    out.append(nop)
                changed = True
            out.append(inst)
        if changed:
            blk.instructions = out


def build_bass(split_waits=True):
    nc = bass.Bass()
    xf = nc.declare_dram_parameter("xf", [65, N], BF16, isOutput=False)
    xq = nc.declare_dram_parameter("xq", [65, NI], BF16, isOutput=False)
    m = nc.declare_dram_parameter("m", [65, C], BF16, isOutput=False)
    wv = nc.declare_dram_parameter("wv", [65, 65], BF16, isOutput=False)
    xres = nc.declare_dram_parameter("xres", [C, NI], F32, isOutput=False)
    onesv = nc.declare_dram_parameter("onesv", [1, C], F32R, isOutput=False)
    y = nc.declare_dram_parameter("y", [C, NI], F32, isOutput=True)

    with tile_mod.TileContext(nc) as tc:
        with (
            tc.tile_pool(name="singles", bufs=1) as singles,
            tc.tile_pool(name="ppool", bufs=3) as ppool,
            tc.tile_pool(name="ypool", bufs=2) as ypool,
            tc.tile_pool(name="small", bufs=4) as small,
            tc.tile_pool(name="ps_et", bufs=2, space="PSUM") as ps_et,
            tc.tile_pool(name="ps_pv", bufs=2, space="PSUM") as ps_pv,
            tc.tile_pool(name="ps_sm", bufs=2, space="PSUM") as ps_sm,
        ):
            # ---- clock warmup: the PE runs at half clock until it sees a
            # bf16 matmul with 256B/col moving data; a short dummy burst on
            # scratch SBUF flips it to 2.4 GHz for the whole program -------
            scratch = singles.tile([128, CHUNK], BF16)
            nc.gpsimd.memset(scratch, 0.0)
            for w in range(6):
                wm = ps_sm.tile([128, CHUNK], F32, tag="sm")
                nc.tensor.matmul(
                    out=wm, lhsT=scratch[:, 0:128], rhs=scratch,
                    start=True, stop=True,
                )

            # ---- load constants and inputs -------------------------------
            m_sb = singles.tile([65, C], BF16)
            nc.gpsimd.dma_start(out=m_sb, in_=m[:, :])
            xq_sb = singles.tile([65, NI], BF16)
            for k in range(NCH):
                ks = slice(k * CHUNK, (k + 1) * CHUNK)
                nc.gpsimd.dma_start(out=xq_sb[:, ks], in_=xq[:, ks])
            wv_sb = singles.tile([65, 65], BF16)
            nc.gpsimd.dma_start(out=wv_sb, in_=wv[:, :])
            xf_sb = singles.tile([65, N], BF16)
            for k in range(8):
                ks = slice(k * (N // 8), (k + 1) * (N // 8))
                nc.sync.dma_start(out=xf_sb[:, ks], in_=xf[:, ks])
            xres_sb = singles.tile([C, NI], F32)
            for k in range(NCH):
                ks = slice(k * CHUNK, (k + 1) * CHUNK)
                nc.gpsimd.dma_start(out=xres_sb[:, ks], in_=xres[:, ks])
            ones_sb = singles.tile([1, C], F32R)
            nc.gpsimd.dma_start(out=ones_sb, in_=onesv[:, :])
            ebias = singles.tile([128, 1], F32)
            nc.gpsimd.memset(ebias, -E0)

            # ---- qk prep: qk = M^T xq + bqk (bf16) -----------------------
            qk_bf = singles.tile([C, NCH, CHUNK], BF16)
            for ch in range(NCH):
                isl = slice(ch * CHUNK, (ch + 1) * CHUNK)
                qs = ps_sm.tile([C, CHUNK], F32, tag="sm")
                nc.tensor.matmul(
                    out=qs, lhsT=m_sb, rhs=xq_sb[:, isl], start=True, stop=True
                )
                nc.scalar.activation(out=qk_bf[:, ch, :], in_=qs, func=COPY)

            # ---- V prep: v'[j, c] fp8, col 64 = row-sum ones -------------
            v_f8 = singles.tile([JB, NJB, 65], F8)
            for jp in range(NJB // 2):
                vp = ps_et.tile([JB, 2, CHUNK], F32, tag="e")
                for g in range(2):
                    jb = 2 * jp + g
                    nc.tensor.matmul(
                        out=vp[:, g, 0:65],
                        lhsT=xf_sb[:, jb * JB : (jb + 1) * JB],
                        rhs=wv_sb,
                        start=True, stop=True,
                    )
                if jp % 2 == 0:
                    nc.scalar.activation(
                        out=v_f8[:, 2 * jp : 2 * jp + 2, :], in_=vp[:, :, 0:65],
                        func=COPY,
                    )
                else:
                    nc.vector.tensor_copy(
                        v_f8[:, 2 * jp : 2 * jp + 2, :], vp[:, :, 0:65]
                    )

            # ---- main: half-passes over chunk pairs, j-block outer -------
            sch_a = float(A_SCH)
            sch_b = float(56.0 - A_SCH * E0 + C_SCH)
            for half in range(2):
                pvs = []
                for cc in range(2):
                    pv = ps_pv.tile([65, CHUNK], F32, tag="pv")
                    pvs.append(pv)
                for jb in range(NJB):
                    et = ps_et.tile([JB, 2, CHUNK], F32, tag="e")
                    for cc in range(2):
                        ch = 2 * half + cc
                        nc.tensor.matmul(
                            out=et[:, cc, :],
                            lhsT=xf_sb[0:C, jb * JB : (jb + 1) * JB],
                            rhs=qk_bf[:, ch, :],
                            start=True, stop=True,
                        )
                    p_t = ppool.tile([JB, 2, CHUNK], F8)
                    if jb % 2 == 0:
                        nc.scalar.activation(
                            out=p_t, in_=et, func=EXP, bias=ebias,
                        )
                    else:
                        nc.vector.tensor_scalar(
                            out=p_t.bitcast(U8), in0=et,
                            scalar1=sch_a, scalar2=sch_b,
                            op0=MULT, op1=ADD,
                        )
                    for cc in range(2):
                        nc.tensor.matmul(
                            out=pvs[cc], lhsT=v_f8[:, jb, :], rhs=p_t[:, cc, :],
                            start=(jb == 0), stop=(jb == NJB - 1),
                        )

                # ---- epilogue: y = pv[0:64]*(1/16 (x) 1/s) + xres --------
                for cc in range(2):
                    ch = 2 * half + cc
                    isl = slice(ch * CHUNK, (ch + 1) * CHUNK)
                    pv = pvs[cc]
                    r_t = small.tile([1, CHUNK], F32R)
                    with nc.allow_low_precision(reason="softmax recip"):
                        nc.vector.reciprocal(out=r_t, in_=pv[64:65, :])
                    lrb = ps_sm.tile([C, CHUNK], F32, tag="sm")
                    nc.tensor.matmul(
                        out=lrb, lhsT=ones_sb, rhs=r_t, start=True, stop=True
                    )
                    lrb_sb = small.tile([C, CHUNK], F32)
                    nc.scalar.activation(out=lrb_sb, in_=lrb, func=COPY)
                    y_t = ypool.tile([C, CHUNK], F32)
                    nc.vector.tensor_tensor(
                        out=y_t, in0=pv[0:C, :], in1=lrb_sb, op=MULT
                    )
                    nc.gpsimd.tensor_tensor(
                        out=y_t, in0=y_t, in1=xres_sb[:, isl], op=ADD
                    )
                    nc.sync.dma_start(out=y[:, isl], in_=y_t)

    if split_waits:
        _split_multi_waits(nc)
    return nc


_CACHE = {}


def kernel(**inputs):
    x = np.ascontiguousarray(np.asarray(inputs["x"], dtype=np.float32))
    x_RGB = np.ascontiguousarray(np.asarray(inputs["x_RGB"], dtype=np.float32))
    Wq = np.asarray(inputs["Wq"], dtype=np.float32)
    bq = np.asarray(inputs["bq"], dtype=np.float32)
    Wk = np.asarray(inputs["Wk"], dtype=np.float32)
    Wv = np.asarray(inputs["Wv"], dtype=np.float32)
    bv = np.asarray(inputs["bv"], dtype=np.float32)
    lam = float(np.asarray(inputs["lam"], dtype=np.float32).reshape(-1)[0])

    M = (Wq.T.astype(np.float64) @ Wk.astype(np.float64)).astype(np.float32)
    bqk = (Wk.T.astype(np.float64) @ bq.astype(np.float64)).astype(np.float32)

    m_aug = np.zeros((65, C), np.float32)
    m_aug[:C] = M
    m_aug[C] = bqk

    # wv: [ch_in(+ones), c_out(+s-col)]: v' = 16*lam*(Wv xf + bv); col 64 = 1
    wv_aug = np.zeros((65, 65), np.float32)
    wv_aug[:C, :C] = (V_SCALE * lam) * Wv.T
    wv_aug[C, :C] = (V_SCALE * lam) * bv
    wv_aug[C, C] = 1.0

    xf3 = x.reshape(B, C, N)
    xr3 = x_RGB.reshape(B, C, N)

    if "nc" not in _CACHE:
        _CACHE["nc"] = build_bass()
    nc = _CACHE["nc"]

    in_maps = []
    for core in range(NCORES):
        b, ih = core >> 1, core & 1
        # columns permuted: own query half first (static residual slice)
        xf_aug = np.empty((65, N), np.float32)
        xf_aug[:C, :NI] = xf3[b][:, ih * NI : (ih + 1) * NI]
        xf_aug[:C, NI:] = xf3[b][:, (1 - ih) * NI : (2 - ih) * NI]
        xf_aug[C] = 1.0
        xq_aug = np.empty((65, NI), np.float32)
        xq_aug[:C] = xr3[b][:, ih * NI : (ih + 1) * NI]
        xq_aug[C] = 1.0
        in_maps.append(
            {
                "xf": xf_aug.astype(ml_dtypes.bfloat16),
                "xq": xq_aug.astype(ml_dtypes.bfloat16),
                "m": m_aug.astype(ml_dtypes.bfloat16),
                "wv": wv_aug.astype(ml_dtypes.bfloat16),
                "xres": np.ascontiguousarray(xf_aug[:C, 0:NI].astype(np.float32)),
                "onesv": np.full((1, C), 1.0 / V_SCALE, np.float32),
            }
        )

    from concourse.bass_utils import run_bass_kernel_spmd

    res = run_bass_kernel_spmd(nc, in_maps, list(range(NCORES)))

    out = np.empty((B, C, N), np.float32)
    for core in range(NCORES):
        b, ih = core >> 1, core & 1
        out[b][:, ih * NI : (ih + 1) * NI] = res.results[core]["y"]
    return out.reshape(B, C, HH, WW)
